# revision 25
# baseline (speedup 1.0000x reference)
"""Fused causal-attention block (QKV proj + causal softmax attention + out proj
+ residual + LayerNorm) on 8 Trainium2 NeuronCores.

Sharding: core c -> batch b = c//4, head-group r = c%4 (heads 4r..4r+3,
d' columns 256r..256r+256).  Each core computes Q/K/V for its head group over
its batch's full sequence and flash-style causal attention (no max subtraction
-- scores are O(1)).  The per-core normalized ctx^T [256, 2048] is AllGather'd
across the batch's 4 cores; each core then reads back the gathered ctx^T for
its own 512 output rows (rank-sliced via partition_id), runs the full output
projection, residual and LayerNorm for those rows.  Host reassembles the 8
[512, 1024] slices.

All matmuls run as float32r (full-rate fp32 on the PE); every tensor feeding a
matmul is float32r end-to-end so the BIR verifier sees rounded producers.  The
causal mask on diagonal 128x128 blocks is applied by accumulating a -1e9
upper-triangular bf16 matrix into the scores PSUM via an extra matmul (rhs =
identity).  Softmax denominators come from an all-ones column appended to V.
The two heads of a partition-tile pair compute their K=64 score matmuls
back-to-back at PE base partitions 0/64 (disjoint row groups -> concurrent),
into one shared [128, 2, 512] PSUM tile that a single strided ACT call
exponentiates for both heads.
"""

import numpy as np

B, N, D = 2, 2048, 1024
H, DH = 16, 64
NCORES = 8
HPC = 4          # heads per core
DP = HPC * DH    # 256 d' columns per core
NQ = N // 4      # 512 output rows per core
LN_EPS = 1e-5
NEG = -1e9
GROUPS = [[0, 1, 2, 3], [4, 5, 6, 7]]

_CACHE = {}


def _build(flags):
    """Build+compile the Bacc program. flags = (has_qkv_bias, has_gamma, has_beta)."""
    import concourse.bass as bass
    import concourse.bacc as bacc
    import concourse.tile as tile
    from concourse import mybir
    from contextlib import ExitStack

    has_qkv_bias, has_gamma, has_beta = flags
    f32 = mybir.dt.float32
    f32r = mybir.dt.float32r
    bf16 = mybir.dt.bfloat16
    AF = mybir.ActivationFunctionType
    ALU = mybir.AluOpType

    nc = bacc.Bacc(
        trn_type="TRN2",
        target_bir_lowering=False,
        debug=False,
        num_devices=NCORES,
    )

    xT = nc.dram_tensor("xT", [D, N], f32r, kind="ExternalInput").ap()
    xres = nc.dram_tensor("xres", [NQ, D], f32, kind="ExternalInput").ap()
    wqT = nc.dram_tensor("wqT", [D, DP], f32r, kind="ExternalInput").ap()
    wkT = nc.dram_tensor("wkT", [D, DP], f32r, kind="ExternalInput").ap()
    wvT = nc.dram_tensor("wvT", [D, DP], f32r, kind="ExternalInput").ap()
    woT = nc.dram_tensor("woT", [D, D], f32r, kind="ExternalInput").ap()
    out = nc.dram_tensor("out", [NQ, D], f32, kind="ExternalOutput").ap()
    if has_qkv_bias:
        bqkv = nc.dram_tensor("bqkv", [1, 3, DP], f32r, kind="ExternalInput").ap()
    if has_gamma:
        gamma_d = nc.dram_tensor("gamma", [D], f32, kind="ExternalInput").ap()
    if has_beta:
        beta_d = nc.dram_tensor("beta", [D], f32, kind="ExternalInput").ap()

    # multiplicative causal mask for diagonal blocks: keep k <= q
    # (partition p = k offset, free c = q offset)
    tri_np = np.triu(np.ones((128, 128), np.float32))  # tri[p, c] = (p <= c)
    tri_d = nc.inline_tensor(np.ascontiguousarray(tri_np.T * 0 + tri_np),
                             name="tri_const").ap()

    with tile.TileContext(nc) as tc, ExitStack() as ctx, \
            nc.allow_low_precision(reason="float32r carries full fp32 bits"):
        singles = ctx.enter_context(tc.tile_pool(name="singles", bufs=1))
        qkv_pool = ctx.enter_context(tc.tile_pool(name="qkv", bufs=1))

        # weights, striped k-on-partitions
        wq_sb = singles.tile([128, 8, DP], f32r, tag="wq")
        wk_sb = singles.tile([128, 8, DP], f32r, tag="wk")
        wv_sb = singles.tile([128, 8, DP], f32r, tag="wv")
        nc.sync.dma_start(wq_sb, wqT.rearrange("(ko p) m -> p ko m", p=128))
        nc.scalar.dma_start(wk_sb, wkT.rearrange("(ko p) m -> p ko m", p=128))
        nc.gpsimd.dma_start(wv_sb, wvT.rearrange("(ko p) m -> p ko m", p=128))

        tri_sb = singles.tile([128, 128], f32, tag="tri")
        nc.sync.dma_start(tri_sb, tri_d)

        ones_f32 = singles.tile([128, 64], f32, tag="ones_f32")
        nc.vector.memset(ones_f32, 1.0)
        ones64 = singles.tile([1, 64], f32r, tag="ones64")
        nc.vector.tensor_copy(out=ones64, in_=ones_f32[0:1, :])
        eps_sb = singles.tile([128, 1], f32, tag="eps")
        nc.vector.memset(eps_sb, LN_EPS)
        if has_qkv_bias:
            o512f = singles.tile([1, 512], f32, tag="o512f")
            nc.vector.memset(o512f, 1.0)
            ones512 = singles.tile([1, 512], f32r, tag="ones512")
            nc.vector.tensor_copy(out=ones512, in_=o512f)
            bqkv_sb = singles.tile([1, 3, DP], f32r, tag="bqkv")
            nc.sync.dma_start(bqkv_sb, bqkv)
        if has_gamma:
            gamma_sb = singles.tile([128, D], f32, tag="gamma")
            nc.sync.dma_start(
                gamma_sb,
                bass.AP(tensor=gamma_d.tensor, offset=gamma_d.offset,
                        ap=[[0, 128]] + gamma_d.ap),
            )
        if has_beta:
            beta_sb = singles.tile([128, D], f32, tag="beta")
            nc.sync.dma_start(
                beta_sb,
                bass.AP(tensor=beta_d.tensor, offset=beta_d.offset,
                        ap=[[0, 128]] + beta_d.ap),
            )

        # preload partition id early so the tail's dynamic DMA doesn't pay
        # the register-load latency
        rank_sv = nc.partition_id() % 4

        # persistent activations
        qT_sb = qkv_pool.tile([128, 2, N], f32r, tag="qT")   # Q^T [d'(256), n]
        kT_sb = qkv_pool.tile([128, 2, N], f32r, tag="kT")   # K^T [d'(256), n]
        v_sb = qkv_pool.tile([128, 16, HPC, DH + 1], f32r, tag="v")  # V + ones
        ctx_sb = qkv_pool.tile([128, 2, N], f32r, tag="ctxT")  # normalized ctx^T
        nc.vector.tensor_copy(
            out=v_sb[:, :, :, DH:DH + 1],
            in_=ones_f32.rearrange("p (a b c) -> p a b c", a=16, b=4))

        # ---------------- Phase 1: QKV projections ----------------
        with tc.tile_pool(name="xt", bufs=1) as xt_pool, \
             tc.tile_pool(name="p1qk", bufs=3, space="PSUM") as p1qk, \
             tc.tile_pool(name="p1v", bufs=2, space="PSUM") as p1v:
            xT_sb = xt_pool.tile([128, 8, N], f32r, tag="xT")
            xT_r = xT.rearrange("(ko p) n -> ko p n", p=128)
            # spread the 8MB load across four engine DMA queues
            dma_engs = [nc.sync, nc.scalar, nc.gpsimd]
            for ko in range(8):
                for hf in range(2):
                    dma_engs[(2 * ko + hf) % 3].dma_start(
                        xT_sb[:, ko, 1024 * hf:1024 * (hf + 1)],
                        xT_r[ko][:, 1024 * hf:1024 * (hf + 1)])

            for wsb, dst, bidx in ((wq_sb, qT_sb, 0), (wk_sb, kT_sb, 1)):
                for dt_ in range(2):
                    for nt in range(4):
                        ps = p1qk.tile([128, 512], f32, tag="qk")
                        for ko in range(8):
                            nc.tensor.matmul(
                                ps,
                                lhsT=wsb[:, ko, 128 * dt_:128 * dt_ + 128],
                                rhs=xT_sb[:, ko, 512 * nt:512 * nt + 512],
                                start=(ko == 0),
                                stop=(ko == 7 and not has_qkv_bias),
                            )
                        if has_qkv_bias:
                            nc.tensor.matmul(
                                ps,
                                lhsT=bqkv_sb[:, bidx, 128 * dt_:128 * dt_ + 128],
                                rhs=ones512,
                                start=False, stop=True,
                            )
                        nc.vector.tensor_copy(
                            out=dst[:, dt_, 512 * nt:512 * (nt + 1)], in_=ps)

            for nt in range(16):
                ps = p1v.tile([128, DP], f32, tag="v")
                for ko in range(8):
                    nc.tensor.matmul(
                        ps,
                        lhsT=xT_sb[:, ko, 128 * nt:128 * nt + 128],
                        rhs=wv_sb[:, ko],
                        start=(ko == 0),
                        stop=(ko == 7 and not has_qkv_bias),
                    )
                if has_qkv_bias:
                    nc.tensor.matmul(
                        ps,
                        lhsT=ones512[:, 0:128],
                        rhs=bqkv_sb[:, 2, :],
                        start=False, stop=True,
                    )
                nc.vector.tensor_copy(
                    out=v_sb[:, nt, :, 0:DH],
                    in_=ps.rearrange("p (h d) -> p h d", h=HPC))

        # full Wo^T: loaded here so the 4MB DMA overlaps attention
        wo_pool = ctx.enter_context(tc.tile_pool(name="wop", bufs=1))
        wo_sb = wo_pool.tile([128, 8, D], f32r, tag="wo")
        nc.sync.dma_start(wo_sb, woT.rearrange("(ko p) m -> p ko m", p=128))

        dram_pool = ctx.enter_context(tc.tile_pool(name="dram", bufs=1,
                                                   space="DRAM"))
        ctxq_dram = [dram_pool.tile([DP, NQ], f32r, tag=f"ctxq{qt}",
                                    name=f"ctxq{qt}")
                     for qt in range(4)]
        # [qt, global d', n-within-qt]; AG(qt) fills slice qt
        ctxg_dram = dram_pool.tile([4, 4 * DP, NQ], f32r, tag="ctxg_dram")

        # ---------------- Phase 2: causal attention ----------------
        # Pair hp owns heads (2hp, 2hp+1) at partition bases 0/64 of qT/kT
        # tile hp.  Per k-tile, both heads' scores land in one [128, 2, 512]
        # PSUM tile (concurrent row-group matmuls), one strided ACT exp call
        # covers both.
        with tc.tile_pool(name="es", bufs=8) as es_pool, \
             tc.tile_pool(name="nrm", bufs=4) as nrm_pool, \
             tc.tile_pool(name="sps", bufs=2, space="PSUM") as sps_pool, \
             tc.tile_pool(name="cps", bufs=2, space="PSUM") as cps_pool:
            def emit_recip(qt, hp, cps):
                # 1/denom with the 512-vector spread over 64 partitions via
                # DMA reshape, so the DVE reciprocal is ~0.2us instead of
                # 2.7us (free-dim-serial) and stays off the critical path.
                recs = []
                for hi, h in enumerate((2 * hp, 2 * hp + 1)):
                    rec = nrm_pool.tile([1, 512], f32r, tag="rec",
                                        name=f"rec_{qt}_{h}")
                    nc.vector.reciprocal(out=rec, in_=cps[64:65, hi, :])
                    recs.append(rec)
                return recs

            def emit_normalize(qt, hp, cps, recs):
                for hi, h in enumerate((2 * hp, 2 * hp + 1)):
                    ph = 64 * (h % 2)
                    bc_full = sps_pool.tile([128, 2, 512], f32, tag="s",
                                            name=f"bc_{qt}_{h}")
                    bc = bc_full[0:64, 0, :]
                    nc.tensor.matmul(bc, lhsT=ones64, rhs=recs[hi],
                                     start=True, stop=True)
                    bcs = nrm_pool.tile([64, 512], f32, tag="bcs",
                                        name=f"bcs_{qt}_{h}")
                    nc.scalar.copy(out=bcs, in_=bc)
                    nc.vector.tensor_mul(
                        out=ctx_sb[ph:ph + 64, hp, 512 * qt:512 * (qt + 1)],
                        in0=cps[0:64, hi, :], in1=bcs)
                if hp == 1:
                    # stage this q-tile's ctx^T and gather the batch's 4
                    # shards; the AllGathers overlap later attention.
                    nc.gpsimd.dma_start(
                        ctxq_dram[qt].rearrange("(t p) n -> p t n", p=128),
                        ctx_sb[:, :, 512 * qt:512 * (qt + 1)])
                    nc.gpsimd.collective_compute(
                        "AllGather", ALU.bypass,
                        replica_groups=GROUPS,
                        ins=[ctxq_dram[qt][:, :]],
                        outs=[ctxg_dram[qt]],
                    )

            pending_norm = None  # (qt, hp, cps) awaiting emission
            for qt in range(4):
                for hp in range(2):
                    heads = (2 * hp, 2 * hp + 1)
                    cps = cps_pool.tile([128, 2, 512], f32, tag="ctx",
                                        name=f"cps_{qt}_{hp}")
                    n_kt = 4 * qt + 4
                    pend = []
                    for kt in range(n_kt):
                        j = kt - 4 * qt
                        c0 = 128 * j if j > 0 else 0
                        sp = sps_pool.tile([128, 2, 512], f32, tag="s",
                                           name=f"sp_{qt}_{hp}_{kt}")
                        for hi, h in enumerate(heads):
                            ph = 64 * (h % 2)
                            nc.tensor.matmul(
                                sp[:, hi],
                                lhsT=kT_sb[ph:ph + 64, hp,
                                           128 * kt:128 * kt + 128],
                                rhs=qT_sb[ph:ph + 64, hp,
                                          512 * qt:512 * (qt + 1)],
                                start=True,
                                stop=True,
                            )
                        es = es_pool.tile([128, 2, 512], f32r, tag="es")
                        nc.scalar.activation(
                            out=es[:, :, c0:512],
                            in_=sp[:, :, c0:512],
                            func=AF.Exp, scale=0.125,
                        )
                        if j >= 0:
                            for hi in range(2):
                                nc.vector.tensor_mul(
                                    out=es[:, hi, 128 * j:128 * j + 128],
                                    in0=es[:, hi, 128 * j:128 * j + 128],
                                    in1=tri_sb)
                        pend.append((es, kt, c0))
                        if len(pend) > 3:
                            _emit_av(nc, cps, v_sb, heads, pend.pop(0), n_kt)
                        # the previous pair's normalize rides in here so the
                        # in-order PE never drains at a pair boundary
                        if kt == min(3, n_kt - 1) and pending_norm is not None:
                            emit_normalize(*pending_norm)
                            pending_norm = None
                    while pend:
                        _emit_av(nc, cps, v_sb, heads, pend.pop(0), n_kt)
                    recs = emit_recip(qt, hp, cps)
                    pending_norm = (qt, hp, cps, recs)
            emit_normalize(*pending_norm)

        # ---------- Phase 3: rank-sliced out proj + residual + LN ----------
        with tc.tile_pool(name="p3sb", bufs=3) as p3sb, \
             tc.tile_pool(name="p3ps", bufs=2, space="PSUM") as p3ps:
            # my output rows live in q-tile `rank`'s gather; fetch as 8
            # per-chunk DMAs so the out-proj can start on chunk 0 early
            ctxg_sb = wo_pool.tile([128, 8, NQ], f32r, tag="ctxg")
            ctxg_r = ctxg_dram[bass.ds(rank_sv, 1)].rearrange(
                "a (po p) n -> p (a po) n", p=128)
            for po in range(8):
                nc.sync.dma_start(ctxg_sb[:, po], ctxg_r[:, po])

            for ntl in range(4):
                yt = p3sb.tile([128, D], f32, tag="yt")
                for Dt in range(2):
                    ps = p3ps.tile([128, 512], f32, tag="wops")
                    for po in range(8):
                        nc.tensor.matmul(
                            ps,
                            lhsT=ctxg_sb[:, po, 128 * ntl:128 * ntl + 128],
                            rhs=wo_sb[:, po, 512 * Dt:512 * Dt + 512],
                            start=(po == 0), stop=(po == 7),
                        )
                    nc.scalar.copy(out=yt[:, 512 * Dt:512 * (Dt + 1)], in_=ps)
                rt = p3sb.tile([128, D], f32, tag="rt")
                nc.sync.dma_start(rt, xres[128 * ntl:128 * (ntl + 1)])
                nc.vector.tensor_add(out=yt, in0=yt, in1=rt)
                st = p3sb.tile([128, 2, 6], f32, tag="st")
                nc.vector.bn_stats(out=st[:, 0], in_=yt[:, 0:512])
                nc.vector.bn_stats(out=st[:, 1], in_=yt[:, 512:1024])
                mv = p3sb.tile([128, 2], f32, tag="mv")
                nc.vector.bn_aggr(out=mv, in_=st)
                rstd = p3sb.tile([128, 1], f32, tag="rstd")
                nc.scalar.activation(out=rstd, in_=mv[:, 1:2], func=AF.Sqrt,
                                     bias=eps_sb, scale=1.0)
                nc.vector.reciprocal(out=rstd, in_=rstd)
                nc.vector.tensor_scalar(
                    out=yt, in0=yt, scalar1=mv[:, 0:1], scalar2=rstd,
                    op0=ALU.subtract, op1=ALU.mult)
                if has_gamma:
                    nc.vector.tensor_mul(out=yt, in0=yt, in1=gamma_sb)
                if has_beta:
                    nc.vector.tensor_add(out=yt, in0=yt, in1=beta_sb)
                nc.sync.dma_start(out[128 * ntl:128 * (ntl + 1)], yt)

    nc.compile()
    return nc


def _emit_av(nc, cps, v_sb, heads, pend_item, n_kt):
    es, kt, c0 = pend_item
    for hi, h in enumerate(heads):
        nc.tensor.matmul(
            cps[0:65, hi, c0:512],
            lhsT=v_sb[:, kt, h, :],
            rhs=es[:, hi, c0:512],
            start=(kt == 0),
            stop=(kt == n_kt - 1),
        )


def build_nc(flags=(False, False, False)):
    if flags not in _CACHE:
        _CACHE[flags] = _build(flags)
    return _CACHE[flags]


def make_in_maps(inputs):
    x = np.ascontiguousarray(np.asarray(inputs["x"], dtype=np.float32))
    Wq = np.asarray(inputs["Wq"], np.float32)
    Wk = np.asarray(inputs["Wk"], np.float32)
    Wv = np.asarray(inputs["Wv"], np.float32)
    Wo = np.asarray(inputs["Wo"], np.float32)
    bq = np.asarray(inputs["bq"], np.float32)
    bk = np.asarray(inputs["bk"], np.float32)
    bv = np.asarray(inputs["bv"], np.float32)
    bo = np.asarray(inputs["bo"], np.float32)
    gamma = np.asarray(inputs["ln_gamma"], np.float32)
    beta = np.asarray(inputs["ln_beta"], np.float32)

    has_qkv_bias = bool(np.any(bq) or np.any(bk) or np.any(bv))
    has_gamma = not np.allclose(gamma, 1.0)
    has_beta = bool(np.any(beta))
    flags = (has_qkv_bias, has_gamma, has_beta)

    xres_full = x + bo  # residual with output bias folded in
    WoT = np.ascontiguousarray(Wo.T)

    in_maps = []
    for c in range(NCORES):
        b, r = c // 4, c % 4
        cols = slice(DP * r, DP * (r + 1))
        m = {
            "xT": np.ascontiguousarray(x[b].T),
            "xres": np.ascontiguousarray(xres_full[b, NQ * r:NQ * (r + 1)]),
            "wqT": np.ascontiguousarray(Wq[cols, :].T),
            "wkT": np.ascontiguousarray(Wk[cols, :].T),
            "wvT": np.ascontiguousarray(Wv[cols, :].T),
            "woT": WoT,
        }
        if has_qkv_bias:
            m["bqkv"] = np.ascontiguousarray(
                np.stack([bq[cols], bk[cols], bv[cols]])[None])
        if has_gamma:
            m["gamma"] = gamma
        if has_beta:
            m["beta"] = beta
        in_maps.append(m)
    return flags, in_maps


def assemble(results):
    """results: list of per-core dicts with 'out' [512, 1024]."""
    full = np.empty((B, N, D), dtype=np.float32)
    for c in range(NCORES):
        b, r = c // 4, c % 4
        full[b, NQ * r:NQ * (r + 1)] = results[c]["out"]
    return full


def kernel(**inputs):
    from concourse.bass_utils import run_bass_kernel_spmd

    flags, in_maps = make_in_maps(inputs)
    nc = build_nc(flags)
    res = run_bass_kernel_spmd(nc, in_maps, core_ids=list(range(NCORES)))
    return assemble(res.results)


# revision 26
# speedup vs baseline: 1.0919x; 1.0919x over previous
"""Fused causal-attention block (QKV proj + causal softmax attention + out proj
+ residual + LayerNorm) on 8 Trainium2 NeuronCores.

Sharding: core c -> batch b = c//4, head-group r = c%4 (heads 4r..4r+3,
d' columns 256r..256r+256).  Each core computes Q/K/V for its head group over
its batch's full sequence and flash-style causal attention (no max subtraction
-- scores are O(1)).  The per-core normalized ctx^T [256, 2048] is AllGather'd
across the batch's 4 cores; each core then reads back the gathered ctx^T for
its own 512 output rows (rank-sliced via partition_id), runs the full output
projection, residual and LayerNorm for those rows.  Host reassembles the 8
[512, 1024] slices.

All matmuls run as float32r (full-rate fp32 on the PE); every tensor feeding a
matmul is float32r end-to-end so the BIR verifier sees rounded producers.  The
causal mask on diagonal 128x128 blocks is applied by accumulating a -1e9
upper-triangular bf16 matrix into the scores PSUM via an extra matmul (rhs =
identity).  Softmax denominators come from an all-ones column appended to V.
The two heads of a partition-tile pair compute their K=64 score matmuls
back-to-back at PE base partitions 0/64 (disjoint row groups -> concurrent),
into one shared [128, 2, 512] PSUM tile that a single strided ACT call
exponentiates for both heads.
"""

import numpy as np

B, N, D = 2, 2048, 1024
H, DH = 16, 64
NCORES = 8
HPC = 4          # heads per core
DP = HPC * DH    # 256 d' columns per core
NQ = N // 4      # 512 output rows per core
LN_EPS = 1e-5
NEG = -1e9
GROUPS = [[0, 1, 2, 3], [4, 5, 6, 7]]

_CACHE = {}


def _build(flags):
    """Build+compile the Bacc program. flags = (has_qkv_bias, has_gamma, has_beta)."""
    import concourse.bass as bass
    import concourse.bacc as bacc
    import concourse.tile as tile
    from concourse import mybir
    from contextlib import ExitStack

    has_qkv_bias, has_gamma, has_beta = flags
    f32 = mybir.dt.float32
    f32r = mybir.dt.float32r
    bf16 = mybir.dt.bfloat16
    AF = mybir.ActivationFunctionType
    ALU = mybir.AluOpType

    nc = bacc.Bacc(
        trn_type="TRN2",
        target_bir_lowering=False,
        debug=False,
        num_devices=NCORES,
    )

    xT = nc.dram_tensor("xT", [D, N], f32r, kind="ExternalInput").ap()
    xres = nc.dram_tensor("xres", [NQ, D], f32, kind="ExternalInput").ap()
    wqT = nc.dram_tensor("wqT", [D, DP], f32r, kind="ExternalInput").ap()
    wkT = nc.dram_tensor("wkT", [D, DP], f32r, kind="ExternalInput").ap()
    wvT = nc.dram_tensor("wvT", [D, DP], f32r, kind="ExternalInput").ap()
    woT = nc.dram_tensor("woT", [D, D], f32r, kind="ExternalInput").ap()
    out = nc.dram_tensor("out", [NQ, D], f32, kind="ExternalOutput").ap()
    if has_qkv_bias:
        bqkv = nc.dram_tensor("bqkv", [1, 3, DP], f32r, kind="ExternalInput").ap()
    if has_gamma:
        gamma_d = nc.dram_tensor("gamma", [D], f32, kind="ExternalInput").ap()
    if has_beta:
        beta_d = nc.dram_tensor("beta", [D], f32, kind="ExternalInput").ap()

    # multiplicative causal mask for diagonal blocks: keep k <= q
    # (partition p = k offset, free c = q offset)
    tri_np = np.triu(np.ones((128, 128), np.float32))  # tri[p, c] = (p <= c)
    tri_d = nc.inline_tensor(np.ascontiguousarray(tri_np.T * 0 + tri_np),
                             name="tri_const").ap()

    with tile.TileContext(nc) as tc, ExitStack() as ctx, \
            nc.allow_low_precision(reason="float32r carries full fp32 bits"):
        singles = ctx.enter_context(tc.tile_pool(name="singles", bufs=1))
        qkv_pool = ctx.enter_context(tc.tile_pool(name="qkv", bufs=1))

        # weights, striped k-on-partitions
        wq_sb = singles.tile([128, 8, DP], f32r, tag="wq")
        wk_sb = singles.tile([128, 8, DP], f32r, tag="wk")
        wv_sb = singles.tile([128, 8, DP], f32r, tag="wv")
        nc.sync.dma_start(wq_sb, wqT.rearrange("(ko p) m -> p ko m", p=128))
        nc.scalar.dma_start(wk_sb, wkT.rearrange("(ko p) m -> p ko m", p=128))
        nc.gpsimd.dma_start(wv_sb, wvT.rearrange("(ko p) m -> p ko m", p=128))

        tri_sb = singles.tile([128, 128], f32, tag="tri")
        nc.sync.dma_start(tri_sb, tri_d)

        ones_f32 = singles.tile([128, 64], f32, tag="ones_f32")
        nc.vector.memset(ones_f32, 1.0)
        ones64 = singles.tile([1, 64], f32r, tag="ones64")
        nc.vector.tensor_copy(out=ones64, in_=ones_f32[0:1, :])
        eps_sb = singles.tile([128, 1], f32, tag="eps")
        nc.vector.memset(eps_sb, LN_EPS)
        if has_qkv_bias:
            o512f = singles.tile([1, 512], f32, tag="o512f")
            nc.vector.memset(o512f, 1.0)
            ones512 = singles.tile([1, 512], f32r, tag="ones512")
            nc.vector.tensor_copy(out=ones512, in_=o512f)
            bqkv_sb = singles.tile([1, 3, DP], f32r, tag="bqkv")
            nc.sync.dma_start(bqkv_sb, bqkv)
        if has_gamma:
            gamma_sb = singles.tile([128, D], f32, tag="gamma")
            nc.sync.dma_start(
                gamma_sb,
                bass.AP(tensor=gamma_d.tensor, offset=gamma_d.offset,
                        ap=[[0, 128]] + gamma_d.ap),
            )
        if has_beta:
            beta_sb = singles.tile([128, D], f32, tag="beta")
            nc.sync.dma_start(
                beta_sb,
                bass.AP(tensor=beta_d.tensor, offset=beta_d.offset,
                        ap=[[0, 128]] + beta_d.ap),
            )

        # preload partition id early so the tail's dynamic DMA doesn't pay
        # the register-load latency
        rank_sv = nc.partition_id() % 4

        # persistent activations
        qT_sb = qkv_pool.tile([128, 2, N], f32r, tag="qT")   # Q^T [d'(256), n]
        kT_sb = qkv_pool.tile([128, 2, N], f32r, tag="kT")   # K^T [d'(256), n]
        v_sb = qkv_pool.tile([128, 16, HPC, DH + 1], f32r, tag="v")  # V + ones
        ctx_sb = qkv_pool.tile([128, 2, N], f32r, tag="ctxT")  # normalized ctx^T
        nc.vector.tensor_copy(
            out=v_sb[:, :, :, DH:DH + 1],
            in_=ones_f32.rearrange("p (a b c) -> p a b c", a=16, b=4))

        # ---------------- Phase 1: QKV projections ----------------
        with tc.tile_pool(name="xt", bufs=1) as xt_pool, \
             tc.tile_pool(name="p1qk", bufs=3, space="PSUM") as p1qk, \
             tc.tile_pool(name="p1v", bufs=2, space="PSUM") as p1v:
            xT_sb = xt_pool.tile([128, 8, N], f32r, tag="xT")
            xT_r = xT.rearrange("(ko p) n -> ko p n", p=128)
            # spread the 8MB load across four engine DMA queues
            dma_engs = [nc.sync, nc.scalar, nc.gpsimd]
            for ko in range(8):
                for hf in range(2):
                    dma_engs[(2 * ko + hf) % 3].dma_start(
                        xT_sb[:, ko, 1024 * hf:1024 * (hf + 1)],
                        xT_r[ko][:, 1024 * hf:1024 * (hf + 1)])

            for wsb, dst, bidx in ((wq_sb, qT_sb, 0), (wk_sb, kT_sb, 1)):
                for dt_ in range(2):
                    for nt in range(4):
                        ps = p1qk.tile([128, 512], f32, tag="qk")
                        for ko in range(8):
                            nc.tensor.matmul(
                                ps,
                                lhsT=wsb[:, ko, 128 * dt_:128 * dt_ + 128],
                                rhs=xT_sb[:, ko, 512 * nt:512 * nt + 512],
                                start=(ko == 0),
                                stop=(ko == 7 and not has_qkv_bias),
                            )
                        if has_qkv_bias:
                            nc.tensor.matmul(
                                ps,
                                lhsT=bqkv_sb[:, bidx, 128 * dt_:128 * dt_ + 128],
                                rhs=ones512,
                                start=False, stop=True,
                            )
                        nc.vector.tensor_copy(
                            out=dst[:, dt_, 512 * nt:512 * (nt + 1)], in_=ps)

            for nt in range(16):
                ps = p1v.tile([128, DP], f32, tag="v")
                for ko in range(8):
                    nc.tensor.matmul(
                        ps,
                        lhsT=xT_sb[:, ko, 128 * nt:128 * nt + 128],
                        rhs=wv_sb[:, ko],
                        start=(ko == 0),
                        stop=(ko == 7 and not has_qkv_bias),
                    )
                if has_qkv_bias:
                    nc.tensor.matmul(
                        ps,
                        lhsT=ones512[:, 0:128],
                        rhs=bqkv_sb[:, 2, :],
                        start=False, stop=True,
                    )
                nc.vector.tensor_copy(
                    out=v_sb[:, nt, :, 0:DH],
                    in_=ps.rearrange("p (h d) -> p h d", h=HPC))

        # full Wo^T: loaded here so the 4MB DMA overlaps attention
        wo_pool = ctx.enter_context(tc.tile_pool(name="wop", bufs=1))
        wo_sb = wo_pool.tile([128, 8, D], f32r, tag="wo")
        nc.sync.dma_start(wo_sb, woT.rearrange("(ko p) m -> p ko m", p=128))

        dram_pool = ctx.enter_context(tc.tile_pool(name="dram", bufs=1,
                                                   space="DRAM"))
        ctxq_dram = [dram_pool.tile([DP, NQ], f32r, tag=f"ctxq{qt}",
                                    name=f"ctxq{qt}")
                     for qt in range(4)]
        # [qt, global d', n-within-qt]; AG(qt) fills slice qt
        ctxg_dram = dram_pool.tile([4, 4 * DP, NQ], f32r, tag="ctxg_dram")

        # ---------------- Phase 2: causal attention ----------------
        # Pair hp owns heads (2hp, 2hp+1) at partition bases 0/64 of qT/kT
        # tile hp.  Per k-tile, both heads' scores land in one [128, 2, 512]
        # PSUM tile (concurrent row-group matmuls), one strided ACT exp call
        # covers both.
        with tc.tile_pool(name="es", bufs=6) as es_pool, \
             tc.tile_pool(name="nrm", bufs=4) as nrm_pool, \
             tc.tile_pool(name="sps", bufs=2, space="PSUM") as sps_pool, \
             tc.tile_pool(name="cps", bufs=2, space="PSUM") as cps_pool:
            def emit_recip(qt, hp, cps):
                # 1/denom with the 512-vector spread over 64 partitions via
                # DMA reshape, so the DVE reciprocal is ~0.2us instead of
                # 2.7us (free-dim-serial) and stays off the critical path.
                recs = []
                for hi, h in enumerate((2 * hp, 2 * hp + 1)):
                    rec = nrm_pool.tile([1, 512], f32r, tag="rec",
                                        name=f"rec_{qt}_{h}")
                    nc.vector.reciprocal(out=rec, in_=cps[64:65, hi, :])
                    recs.append(rec)
                return recs

            def emit_normalize(qt, hp, cps, recs):
                for hi, h in enumerate((2 * hp, 2 * hp + 1)):
                    ph = 64 * (h % 2)
                    bc_full = sps_pool.tile([128, 2, 512], f32, tag="s",
                                            name=f"bc_{qt}_{h}")
                    bc = bc_full[0:64, 0, :]
                    nc.tensor.matmul(bc, lhsT=ones64, rhs=recs[hi],
                                     start=True, stop=True)
                    bcs = nrm_pool.tile([64, 512], f32, tag="bcs",
                                        name=f"bcs_{qt}_{h}")
                    nc.vector.tensor_copy(out=bcs, in_=bc)
                    nc.vector.tensor_mul(
                        out=ctx_sb[ph:ph + 64, hp, 512 * qt:512 * (qt + 1)],
                        in0=cps[0:64, hi, :], in1=bcs)
                if hp == 1:
                    # stage this q-tile's ctx^T and gather the batch's 4
                    # shards; the AllGathers overlap later attention.
                    nc.gpsimd.dma_start(
                        ctxq_dram[qt].rearrange("(t p) n -> p t n", p=128),
                        ctx_sb[:, :, 512 * qt:512 * (qt + 1)])
                    nc.gpsimd.collective_compute(
                        "AllGather", ALU.bypass,
                        replica_groups=GROUPS,
                        ins=[ctxq_dram[qt][:, :]],
                        outs=[ctxg_dram[qt]],
                    )

            pending_norm = None  # (qt, hp, cps) awaiting emission
            for qt in range(4):
                for hp in range(2):
                    heads = (2 * hp, 2 * hp + 1)
                    cps = cps_pool.tile([128, 2, 512], f32, tag="ctx",
                                        name=f"cps_{qt}_{hp}")
                    n_kt = 4 * qt + 4
                    pend = []
                    for kt in range(n_kt):
                        j = kt - 4 * qt
                        c0 = 128 * j if j > 0 else 0
                        sp = sps_pool.tile([128, 2, 512], f32, tag="s",
                                           name=f"sp_{qt}_{hp}_{kt}")
                        for hi, h in enumerate(heads):
                            ph = 64 * (h % 2)
                            nc.tensor.matmul(
                                sp[:, hi],
                                lhsT=kT_sb[ph:ph + 64, hp,
                                           128 * kt:128 * kt + 128],
                                rhs=qT_sb[ph:ph + 64, hp,
                                          512 * qt:512 * (qt + 1)],
                                start=True,
                                stop=True,
                            )
                        es = es_pool.tile([128, 2, 512], f32r, tag="es")
                        nc.scalar.activation(
                            out=es[:, :, c0:512],
                            in_=sp[:, :, c0:512],
                            func=AF.Exp, scale=0.125,
                        )
                        if j >= 0:
                            for hi in range(2):
                                nc.vector.tensor_mul(
                                    out=es[:, hi, 128 * j:128 * j + 128],
                                    in0=es[:, hi, 128 * j:128 * j + 128],
                                    in1=tri_sb)
                        pend.append((es, kt, c0))
                        if len(pend) > 2:
                            _emit_av(nc, cps, v_sb, heads, pend.pop(0), n_kt)
                        # the previous pair's normalize rides in here so the
                        # in-order PE never drains at a pair boundary
                        if kt == min(3, n_kt - 1) and pending_norm is not None:
                            emit_normalize(*pending_norm)
                            pending_norm = None
                    while pend:
                        _emit_av(nc, cps, v_sb, heads, pend.pop(0), n_kt)
                    recs = emit_recip(qt, hp, cps)
                    pending_norm = (qt, hp, cps, recs)
            emit_normalize(*pending_norm)

        # ---------- Phase 3: rank-sliced out proj + residual + LN ----------
        with tc.tile_pool(name="p3sb", bufs=3) as p3sb, \
             tc.tile_pool(name="p3ps", bufs=2, space="PSUM") as p3ps:
            # my output rows live in q-tile `rank`'s gather; fetch as 8
            # per-chunk DMAs so the out-proj can start on chunk 0 early
            ctxg_sb = wo_pool.tile([128, 8, NQ], f32r, tag="ctxg")
            ctxg_r = ctxg_dram[bass.ds(rank_sv, 1)].rearrange(
                "a (po p) n -> p (a po) n", p=128)
            for po in range(8):
                nc.sync.dma_start(ctxg_sb[:, po], ctxg_r[:, po])

            for ntl in range(4):
                yt = p3sb.tile([128, D], f32, tag="yt")
                for Dt in range(2):
                    ps = p3ps.tile([128, 512], f32, tag="wops")
                    for po in range(8):
                        nc.tensor.matmul(
                            ps,
                            lhsT=ctxg_sb[:, po, 128 * ntl:128 * ntl + 128],
                            rhs=wo_sb[:, po, 512 * Dt:512 * Dt + 512],
                            start=(po == 0), stop=(po == 7),
                        )
                    nc.scalar.copy(out=yt[:, 512 * Dt:512 * (Dt + 1)], in_=ps)
                rt = p3sb.tile([128, D], f32, tag="rt")
                nc.sync.dma_start(rt, xres[128 * ntl:128 * (ntl + 1)])
                nc.vector.tensor_add(out=yt, in0=yt, in1=rt)
                st = p3sb.tile([128, 2, 6], f32, tag="st")
                nc.vector.bn_stats(out=st[:, 0], in_=yt[:, 0:512])
                nc.vector.bn_stats(out=st[:, 1], in_=yt[:, 512:1024])
                mv = p3sb.tile([128, 2], f32, tag="mv")
                nc.vector.bn_aggr(out=mv, in_=st)
                rstd = p3sb.tile([128, 1], f32, tag="rstd")
                nc.scalar.activation(out=rstd, in_=mv[:, 1:2], func=AF.Sqrt,
                                     bias=eps_sb, scale=1.0)
                nc.vector.reciprocal(out=rstd, in_=rstd)
                nc.vector.tensor_scalar(
                    out=yt, in0=yt, scalar1=mv[:, 0:1], scalar2=rstd,
                    op0=ALU.subtract, op1=ALU.mult)
                if has_gamma:
                    nc.vector.tensor_mul(out=yt, in0=yt, in1=gamma_sb)
                if has_beta:
                    nc.vector.tensor_add(out=yt, in0=yt, in1=beta_sb)
                nc.sync.dma_start(out[128 * ntl:128 * (ntl + 1)], yt)

    nc.compile()
    return nc


def _emit_av(nc, cps, v_sb, heads, pend_item, n_kt):
    es, kt, c0 = pend_item
    for hi, h in enumerate(heads):
        nc.tensor.matmul(
            cps[0:65, hi, c0:512],
            lhsT=v_sb[:, kt, h, :],
            rhs=es[:, hi, c0:512],
            start=(kt == 0),
            stop=(kt == n_kt - 1),
        )


def build_nc(flags=(False, False, False)):
    if flags not in _CACHE:
        _CACHE[flags] = _build(flags)
    return _CACHE[flags]


def make_in_maps(inputs):
    x = np.ascontiguousarray(np.asarray(inputs["x"], dtype=np.float32))
    Wq = np.asarray(inputs["Wq"], np.float32)
    Wk = np.asarray(inputs["Wk"], np.float32)
    Wv = np.asarray(inputs["Wv"], np.float32)
    Wo = np.asarray(inputs["Wo"], np.float32)
    bq = np.asarray(inputs["bq"], np.float32)
    bk = np.asarray(inputs["bk"], np.float32)
    bv = np.asarray(inputs["bv"], np.float32)
    bo = np.asarray(inputs["bo"], np.float32)
    gamma = np.asarray(inputs["ln_gamma"], np.float32)
    beta = np.asarray(inputs["ln_beta"], np.float32)

    has_qkv_bias = bool(np.any(bq) or np.any(bk) or np.any(bv))
    has_gamma = not np.allclose(gamma, 1.0)
    has_beta = bool(np.any(beta))
    flags = (has_qkv_bias, has_gamma, has_beta)

    xres_full = x + bo  # residual with output bias folded in
    WoT = np.ascontiguousarray(Wo.T)

    in_maps = []
    for c in range(NCORES):
        b, r = c // 4, c % 4
        cols = slice(DP * r, DP * (r + 1))
        m = {
            "xT": np.ascontiguousarray(x[b].T),
            "xres": np.ascontiguousarray(xres_full[b, NQ * r:NQ * (r + 1)]),
            "wqT": np.ascontiguousarray(Wq[cols, :].T),
            "wkT": np.ascontiguousarray(Wk[cols, :].T),
            "wvT": np.ascontiguousarray(Wv[cols, :].T),
            "woT": WoT,
        }
        if has_qkv_bias:
            m["bqkv"] = np.ascontiguousarray(
                np.stack([bq[cols], bk[cols], bv[cols]])[None])
        if has_gamma:
            m["gamma"] = gamma
        if has_beta:
            m["beta"] = beta
        in_maps.append(m)
    return flags, in_maps


def assemble(results):
    """results: list of per-core dicts with 'out' [512, 1024]."""
    full = np.empty((B, N, D), dtype=np.float32)
    for c in range(NCORES):
        b, r = c // 4, c % 4
        full[b, NQ * r:NQ * (r + 1)] = results[c]["out"]
    return full


def kernel(**inputs):
    from concourse.bass_utils import run_bass_kernel_spmd

    flags, in_maps = make_in_maps(inputs)
    nc = build_nc(flags)
    res = run_bass_kernel_spmd(nc, in_maps, core_ids=list(range(NCORES)))
    return assemble(res.results)


# revision 27
# speedup vs baseline: 1.0998x; 1.0072x over previous
"""Fused causal-attention block (QKV proj + causal softmax attention + out proj
+ residual + LayerNorm) on 8 Trainium2 NeuronCores.

Sharding: core c -> batch b = c//4, head-group r = c%4 (heads 4r..4r+3,
d' columns 256r..256r+256).  Each core computes Q/K/V for its head group over
its batch's full sequence and flash-style causal attention (no max subtraction
-- scores are O(1)).  The per-core normalized ctx^T [256, 2048] is AllGather'd
across the batch's 4 cores; each core then reads back the gathered ctx^T for
its own 512 output rows (rank-sliced via partition_id), runs the full output
projection, residual and LayerNorm for those rows.  Host reassembles the 8
[512, 1024] slices.

All matmuls run as float32r (full-rate fp32 on the PE); every tensor feeding a
matmul is float32r end-to-end so the BIR verifier sees rounded producers.  The
causal mask on diagonal 128x128 blocks is applied by accumulating a -1e9
upper-triangular bf16 matrix into the scores PSUM via an extra matmul (rhs =
identity).  Softmax denominators come from an all-ones column appended to V.
The two heads of a partition-tile pair compute their K=64 score matmuls
back-to-back at PE base partitions 0/64 (disjoint row groups -> concurrent),
into one shared [128, 2, 512] PSUM tile that a single strided ACT call
exponentiates for both heads.
"""

import numpy as np

B, N, D = 2, 2048, 1024
H, DH = 16, 64
NCORES = 8
HPC = 4          # heads per core
DP = HPC * DH    # 256 d' columns per core
NQ = N // 4      # 512 output rows per core
LN_EPS = 1e-5
NEG = -1e9
GROUPS = [[0, 1, 2, 3], [4, 5, 6, 7]]

_CACHE = {}


def _build(flags):
    """Build+compile the Bacc program. flags = (has_qkv_bias, has_gamma, has_beta)."""
    import concourse.bass as bass
    import concourse.bacc as bacc
    import concourse.tile as tile
    from concourse import mybir
    from contextlib import ExitStack

    has_qkv_bias, has_gamma, has_beta = flags
    f32 = mybir.dt.float32
    f32r = mybir.dt.float32r
    bf16 = mybir.dt.bfloat16
    AF = mybir.ActivationFunctionType
    ALU = mybir.AluOpType

    nc = bacc.Bacc(
        trn_type="TRN2",
        target_bir_lowering=False,
        debug=False,
        num_devices=NCORES,
    )

    xT = nc.dram_tensor("xT", [D, N], f32r, kind="ExternalInput").ap()
    xres = nc.dram_tensor("xres", [NQ, D], f32, kind="ExternalInput").ap()
    wqT = nc.dram_tensor("wqT", [D, DP], f32r, kind="ExternalInput").ap()
    wkT = nc.dram_tensor("wkT", [D, DP], f32r, kind="ExternalInput").ap()
    wvT = nc.dram_tensor("wvT", [D, DP], f32r, kind="ExternalInput").ap()
    woT = nc.dram_tensor("woT", [D, D], f32r, kind="ExternalInput").ap()
    out = nc.dram_tensor("out", [NQ, D], f32, kind="ExternalOutput").ap()
    if has_qkv_bias:
        bqkv = nc.dram_tensor("bqkv", [1, 3, DP], f32r, kind="ExternalInput").ap()
    if has_gamma:
        gamma_d = nc.dram_tensor("gamma", [D], f32, kind="ExternalInput").ap()
    if has_beta:
        beta_d = nc.dram_tensor("beta", [D], f32, kind="ExternalInput").ap()

    # multiplicative causal mask for diagonal blocks: keep k <= q
    # (partition p = k offset, free c = q offset)
    tri_np = np.triu(np.ones((128, 128), np.float32))  # tri[p, c] = (p <= c)
    tri_d = nc.inline_tensor(np.ascontiguousarray(tri_np.T * 0 + tri_np),
                             name="tri_const").ap()

    with tile.TileContext(nc) as tc, ExitStack() as ctx, \
            nc.allow_low_precision(reason="float32r carries full fp32 bits"):
        singles = ctx.enter_context(tc.tile_pool(name="singles", bufs=1))
        qkv_pool = ctx.enter_context(tc.tile_pool(name="qkv", bufs=1))

        # weights, striped k-on-partitions
        wq_sb = singles.tile([128, 8, DP], f32r, tag="wq")
        wk_sb = singles.tile([128, 8, DP], f32r, tag="wk")
        wv_sb = singles.tile([128, 8, DP], f32r, tag="wv")
        nc.sync.dma_start(wq_sb, wqT.rearrange("(ko p) m -> p ko m", p=128))
        nc.scalar.dma_start(wk_sb, wkT.rearrange("(ko p) m -> p ko m", p=128))
        nc.gpsimd.dma_start(wv_sb, wvT.rearrange("(ko p) m -> p ko m", p=128))

        tri_sb = singles.tile([128, 128], f32, tag="tri")
        nc.sync.dma_start(tri_sb, tri_d)

        ones_f32 = singles.tile([128, 64], f32, tag="ones_f32")
        nc.vector.memset(ones_f32, 1.0)
        ones64 = singles.tile([1, 64], f32r, tag="ones64")
        nc.vector.tensor_copy(out=ones64, in_=ones_f32[0:1, :])
        eps_sb = singles.tile([128, 1], f32, tag="eps")
        nc.vector.memset(eps_sb, LN_EPS)
        if has_qkv_bias:
            o512f = singles.tile([1, 512], f32, tag="o512f")
            nc.vector.memset(o512f, 1.0)
            ones512 = singles.tile([1, 512], f32r, tag="ones512")
            nc.vector.tensor_copy(out=ones512, in_=o512f)
            bqkv_sb = singles.tile([1, 3, DP], f32r, tag="bqkv")
            nc.sync.dma_start(bqkv_sb, bqkv)
        if has_gamma:
            gamma_sb = singles.tile([128, D], f32, tag="gamma")
            nc.sync.dma_start(
                gamma_sb,
                bass.AP(tensor=gamma_d.tensor, offset=gamma_d.offset,
                        ap=[[0, 128]] + gamma_d.ap),
            )
        if has_beta:
            beta_sb = singles.tile([128, D], f32, tag="beta")
            nc.sync.dma_start(
                beta_sb,
                bass.AP(tensor=beta_d.tensor, offset=beta_d.offset,
                        ap=[[0, 128]] + beta_d.ap),
            )

        # preload partition id early so the tail's dynamic DMA doesn't pay
        # the register-load latency
        rank_sv = nc.partition_id() % 4

        # persistent activations
        qT_sb = qkv_pool.tile([128, 2, N], f32r, tag="qT")   # Q^T [d'(256), n]
        kT_sb = qkv_pool.tile([128, 2, N], f32r, tag="kT")   # K^T [d'(256), n]
        v_sb = qkv_pool.tile([128, 16, HPC, DH + 1], f32r, tag="v")  # V + ones
        ctx_sb = qkv_pool.tile([128, 2, N], f32r, tag="ctxT")  # normalized ctx^T
        nc.vector.tensor_copy(
            out=v_sb[:, :, :, DH:DH + 1],
            in_=ones_f32.rearrange("p (a b c) -> p a b c", a=16, b=4))

        # ---------------- Phase 1: QKV projections ----------------
        with tc.tile_pool(name="xt", bufs=1) as xt_pool, \
             tc.tile_pool(name="p1qk", bufs=3, space="PSUM") as p1qk, \
             tc.tile_pool(name="p1v", bufs=2, space="PSUM") as p1v:
            xT_sb = xt_pool.tile([128, 8, N], f32r, tag="xT")
            xT_r = xT.rearrange("(ko p) n -> ko p n", p=128)
            # spread the 8MB load across four engine DMA queues
            dma_engs = [nc.sync, nc.scalar, nc.gpsimd]
            for ko in range(8):
                for hf in range(2):
                    dma_engs[(2 * ko + hf) % 3].dma_start(
                        xT_sb[:, ko, 1024 * hf:1024 * (hf + 1)],
                        xT_r[ko][:, 1024 * hf:1024 * (hf + 1)])

            for wsb, dst, bidx in ((wq_sb, qT_sb, 0), (wk_sb, kT_sb, 1)):
                for dt_ in range(2):
                    for nt in range(4):
                        ps = p1qk.tile([128, 512], f32, tag="qk")
                        for ko in range(8):
                            nc.tensor.matmul(
                                ps,
                                lhsT=wsb[:, ko, 128 * dt_:128 * dt_ + 128],
                                rhs=xT_sb[:, ko, 512 * nt:512 * nt + 512],
                                start=(ko == 0),
                                stop=(ko == 7 and not has_qkv_bias),
                            )
                        if has_qkv_bias:
                            nc.tensor.matmul(
                                ps,
                                lhsT=bqkv_sb[:, bidx, 128 * dt_:128 * dt_ + 128],
                                rhs=ones512,
                                start=False, stop=True,
                            )
                        nc.vector.tensor_copy(
                            out=dst[:, dt_, 512 * nt:512 * (nt + 1)], in_=ps)

            for nt in range(16):
                ps = p1v.tile([128, DP], f32, tag="v")
                for ko in range(8):
                    nc.tensor.matmul(
                        ps,
                        lhsT=xT_sb[:, ko, 128 * nt:128 * nt + 128],
                        rhs=wv_sb[:, ko],
                        start=(ko == 0),
                        stop=(ko == 7 and not has_qkv_bias),
                    )
                if has_qkv_bias:
                    nc.tensor.matmul(
                        ps,
                        lhsT=ones512[:, 0:128],
                        rhs=bqkv_sb[:, 2, :],
                        start=False, stop=True,
                    )
                nc.vector.tensor_copy(
                    out=v_sb[:, nt, :, 0:DH],
                    in_=ps.rearrange("p (h d) -> p h d", h=HPC))

        # full Wo^T: loaded here so the 4MB DMA overlaps attention
        wo_pool = ctx.enter_context(tc.tile_pool(name="wop", bufs=1))
        wo_sb = wo_pool.tile([128, 8, D], f32r, tag="wo")
        nc.sync.dma_start(wo_sb, woT.rearrange("(ko p) m -> p ko m", p=128))

        dram_pool = ctx.enter_context(tc.tile_pool(name="dram", bufs=1,
                                                   space="DRAM"))
        ctxq_dram = [dram_pool.tile([DP, NQ], f32r, tag=f"ctxq{qt}",
                                    name=f"ctxq{qt}")
                     for qt in range(4)]
        # [qt, global d', n-within-qt]; AG(qt) fills slice qt
        ctxg_dram = dram_pool.tile([4, 4 * DP, NQ], f32r, tag="ctxg_dram")

        # ---------------- Phase 2: causal attention ----------------
        # Pair hp owns heads (2hp, 2hp+1) at partition bases 0/64 of qT/kT
        # tile hp.  Per k-tile, both heads' scores land in one [128, 2, 512]
        # PSUM tile (concurrent row-group matmuls), one strided ACT exp call
        # covers both.
        with tc.tile_pool(name="es", bufs=6) as es_pool, \
             tc.tile_pool(name="nrm", bufs=4) as nrm_pool, \
             tc.tile_pool(name="sps", bufs=2, space="PSUM") as sps_pool, \
             tc.tile_pool(name="cps", bufs=2, space="PSUM") as cps_pool:
            def emit_recip(qt, hp, cps):
                # 1/denom with the 512-vector spread over 64 partitions via
                # DMA reshape, so the DVE reciprocal is ~0.2us instead of
                # 2.7us (free-dim-serial) and stays off the critical path.
                recs = []
                for hi, h in enumerate((2 * hp, 2 * hp + 1)):
                    rec = nrm_pool.tile([1, 512], f32r, tag="rec",
                                        name=f"rec_{qt}_{h}")
                    nc.vector.reciprocal(out=rec, in_=cps[64:65, hi, :])
                    recs.append(rec)
                return recs

            def emit_normalize(qt, hp, cps, recs):
                for hi, h in enumerate((2 * hp, 2 * hp + 1)):
                    ph = 64 * (h % 2)
                    bc_full = sps_pool.tile([128, 2, 512], f32, tag="s",
                                            name=f"bc_{qt}_{h}")
                    bc = bc_full[0:64, 0, :]
                    nc.tensor.matmul(bc, lhsT=ones64, rhs=recs[hi],
                                     start=True, stop=True)
                    bcs = nrm_pool.tile([64, 512], f32, tag="bcs",
                                        name=f"bcs_{qt}_{h}")
                    nc.vector.tensor_copy(out=bcs, in_=bc)
                    nc.vector.tensor_mul(
                        out=ctx_sb[ph:ph + 64, hp, 512 * qt:512 * (qt + 1)],
                        in0=cps[0:64, hi, :], in1=bcs)
                if hp == 1:
                    # stage this q-tile's ctx^T and gather the batch's 4
                    # shards; the AllGathers overlap later attention.
                    nc.gpsimd.dma_start(
                        ctxq_dram[qt].rearrange("(t p) n -> p t n", p=128),
                        ctx_sb[:, :, 512 * qt:512 * (qt + 1)])
                    nc.gpsimd.collective_compute(
                        "AllGather", ALU.bypass,
                        replica_groups=GROUPS,
                        ins=[ctxq_dram[qt][:, :]],
                        outs=[ctxg_dram[qt]],
                    )

            pending_norm = None  # (qt, hp, cps) awaiting emission
            for qt in range(4):
                for hp in range(2):
                    heads = (2 * hp, 2 * hp + 1)
                    cps = cps_pool.tile([128, 2, 512], f32, tag="ctx",
                                        name=f"cps_{qt}_{hp}")
                    n_kt = 4 * qt + 4
                    pend = []
                    for kt in range(n_kt):
                        j = kt - 4 * qt
                        c0 = 128 * j if j > 0 else 0
                        sp = sps_pool.tile([128, 2, 512], f32, tag="s",
                                           name=f"sp_{qt}_{hp}_{kt}")
                        for hi, h in enumerate(heads):
                            ph = 64 * (h % 2)
                            nc.tensor.matmul(
                                sp[:, hi],
                                lhsT=kT_sb[ph:ph + 64, hp,
                                           128 * kt:128 * kt + 128],
                                rhs=qT_sb[ph:ph + 64, hp,
                                          512 * qt:512 * (qt + 1)],
                                start=True,
                                stop=True,
                            )
                        es = es_pool.tile([128, 2, 512], f32r, tag="es")
                        nc.scalar.activation(
                            out=es[:, :, c0:512],
                            in_=sp[:, :, c0:512],
                            func=AF.Exp, scale=0.125,
                        )
                        if j >= 0:
                            for hi in range(2):
                                nc.vector.tensor_mul(
                                    out=es[:, hi, 128 * j:128 * j + 128],
                                    in0=es[:, hi, 128 * j:128 * j + 128],
                                    in1=tri_sb)
                        pend.append((es, kt, c0))
                        if len(pend) > 2:
                            _emit_av(nc, cps, v_sb, heads, pend.pop(0), n_kt)
                        # the previous pair's normalize rides in here so the
                        # in-order PE never drains at a pair boundary
                        if kt == min(5, n_kt - 1) and pending_norm is not None:
                            emit_normalize(*pending_norm)
                            pending_norm = None
                    while pend:
                        _emit_av(nc, cps, v_sb, heads, pend.pop(0), n_kt)
                    recs = emit_recip(qt, hp, cps)
                    pending_norm = (qt, hp, cps, recs)
            emit_normalize(*pending_norm)

        # ---------- Phase 3: rank-sliced out proj + residual + LN ----------
        with tc.tile_pool(name="p3sb", bufs=3) as p3sb, \
             tc.tile_pool(name="p3ps", bufs=2, space="PSUM") as p3ps:
            # my output rows live in q-tile `rank`'s gather; fetch as 8
            # per-chunk DMAs so the out-proj can start on chunk 0 early
            ctxg_sb = wo_pool.tile([128, 8, NQ], f32r, tag="ctxg")
            ctxg_r = ctxg_dram[bass.ds(rank_sv, 1)].rearrange(
                "a (po p) n -> p (a po) n", p=128)
            for po in range(8):
                nc.sync.dma_start(ctxg_sb[:, po], ctxg_r[:, po])

            for ntl in range(4):
                yt = p3sb.tile([128, D], f32, tag="yt")
                for Dt in range(2):
                    ps = p3ps.tile([128, 512], f32, tag="wops")
                    for po in range(8):
                        nc.tensor.matmul(
                            ps,
                            lhsT=ctxg_sb[:, po, 128 * ntl:128 * ntl + 128],
                            rhs=wo_sb[:, po, 512 * Dt:512 * Dt + 512],
                            start=(po == 0), stop=(po == 7),
                        )
                    nc.scalar.copy(out=yt[:, 512 * Dt:512 * (Dt + 1)], in_=ps)
                rt = p3sb.tile([128, D], f32, tag="rt")
                nc.sync.dma_start(rt, xres[128 * ntl:128 * (ntl + 1)])
                nc.vector.tensor_add(out=yt, in0=yt, in1=rt)
                st = p3sb.tile([128, 2, 6], f32, tag="st")
                nc.vector.bn_stats(out=st[:, 0], in_=yt[:, 0:512])
                nc.vector.bn_stats(out=st[:, 1], in_=yt[:, 512:1024])
                mv = p3sb.tile([128, 2], f32, tag="mv")
                nc.vector.bn_aggr(out=mv, in_=st)
                rstd = p3sb.tile([128, 1], f32, tag="rstd")
                nc.scalar.activation(out=rstd, in_=mv[:, 1:2], func=AF.Sqrt,
                                     bias=eps_sb, scale=1.0)
                nc.vector.reciprocal(out=rstd, in_=rstd)
                nc.vector.tensor_scalar(
                    out=yt, in0=yt, scalar1=mv[:, 0:1], scalar2=rstd,
                    op0=ALU.subtract, op1=ALU.mult)
                if has_gamma:
                    nc.vector.tensor_mul(out=yt, in0=yt, in1=gamma_sb)
                if has_beta:
                    nc.vector.tensor_add(out=yt, in0=yt, in1=beta_sb)
                nc.sync.dma_start(out[128 * ntl:128 * (ntl + 1)], yt)

    nc.compile()
    return nc


def _emit_av(nc, cps, v_sb, heads, pend_item, n_kt):
    es, kt, c0 = pend_item
    for hi, h in enumerate(heads):
        nc.tensor.matmul(
            cps[0:65, hi, c0:512],
            lhsT=v_sb[:, kt, h, :],
            rhs=es[:, hi, c0:512],
            start=(kt == 0),
            stop=(kt == n_kt - 1),
        )


def build_nc(flags=(False, False, False)):
    if flags not in _CACHE:
        _CACHE[flags] = _build(flags)
    return _CACHE[flags]


def make_in_maps(inputs):
    x = np.ascontiguousarray(np.asarray(inputs["x"], dtype=np.float32))
    Wq = np.asarray(inputs["Wq"], np.float32)
    Wk = np.asarray(inputs["Wk"], np.float32)
    Wv = np.asarray(inputs["Wv"], np.float32)
    Wo = np.asarray(inputs["Wo"], np.float32)
    bq = np.asarray(inputs["bq"], np.float32)
    bk = np.asarray(inputs["bk"], np.float32)
    bv = np.asarray(inputs["bv"], np.float32)
    bo = np.asarray(inputs["bo"], np.float32)
    gamma = np.asarray(inputs["ln_gamma"], np.float32)
    beta = np.asarray(inputs["ln_beta"], np.float32)

    has_qkv_bias = bool(np.any(bq) or np.any(bk) or np.any(bv))
    has_gamma = not np.allclose(gamma, 1.0)
    has_beta = bool(np.any(beta))
    flags = (has_qkv_bias, has_gamma, has_beta)

    xres_full = x + bo  # residual with output bias folded in
    WoT = np.ascontiguousarray(Wo.T)

    in_maps = []
    for c in range(NCORES):
        b, r = c // 4, c % 4
        cols = slice(DP * r, DP * (r + 1))
        m = {
            "xT": np.ascontiguousarray(x[b].T),
            "xres": np.ascontiguousarray(xres_full[b, NQ * r:NQ * (r + 1)]),
            "wqT": np.ascontiguousarray(Wq[cols, :].T),
            "wkT": np.ascontiguousarray(Wk[cols, :].T),
            "wvT": np.ascontiguousarray(Wv[cols, :].T),
            "woT": WoT,
        }
        if has_qkv_bias:
            m["bqkv"] = np.ascontiguousarray(
                np.stack([bq[cols], bk[cols], bv[cols]])[None])
        if has_gamma:
            m["gamma"] = gamma
        if has_beta:
            m["beta"] = beta
        in_maps.append(m)
    return flags, in_maps


def assemble(results):
    """results: list of per-core dicts with 'out' [512, 1024]."""
    full = np.empty((B, N, D), dtype=np.float32)
    for c in range(NCORES):
        b, r = c // 4, c % 4
        full[b, NQ * r:NQ * (r + 1)] = results[c]["out"]
    return full


def kernel(**inputs):
    from concourse.bass_utils import run_bass_kernel_spmd

    flags, in_maps = make_in_maps(inputs)
    nc = build_nc(flags)
    res = run_bass_kernel_spmd(nc, in_maps, core_ids=list(range(NCORES)))
    return assemble(res.results)
